# revision 1
# baseline (speedup 1.0000x reference)
"""MARL halftone REINFORCE loss on 8 Trainium2 NeuronCores.

Math (per batch image, all 512x512):
    e    = G*h - c            (G = 11x11 gaussian, SAME zero pad)
    corr = G*e
    reward = 2*delta*corr + delta^2*K2,  delta = 1-2h in {-1,+1} so delta^2 = 1
    lp   = log(p+eps) if h else log(1-p+eps) = ln|h+p-1| (+O(1e-6))
    loss = -sum_b sum_px (reward*lp) / B

Conv as banded matrix A (A[i,j] = gn[j-i+5], SAME-pad truncation at edges):
    G*x = A x A.   corr = A(AhA - c)A = B h B - A c A,  B = A@A (matrix product,
    edge-exact).  On the PE, op2(X; M) := X^T M, and op2(op2(X; M); M) = M X M
    with no transposes (M symmetric).  So the h-chain and c-chain run as two
    independent 2-pass pipelines.  Matmuls run in float32r (fp22) at full rate
    with 256-wide band windows.

Final reduction:
    sum(reward*lp)/(-8) = 0.5<T2, gt> - 0.5<S2, gt> - (K2/8)*sum(lp)
    where T2 = BhB, S2 = AcA, gt = (h-0.5)*lp = -delta*lp/2.
    <.,.> accumulated per-partition by fused scalar_tensor_tensor accum_out,
    and sum(lp) by the Ln activation's accum_out.  [128, 9] partials are
    fetched per core and summed on the host.

Host/transfer layer (the wall-clock bottleneck is the axon tunnel + RPC
latency, not the device — the on-device kernel is ~20 us):
  - s = +-p bf16 per core [512, 512] with sign bit = 1-h (s = p if h==1
    else -p); h is recovered on-device as h = (s > 0), p via
    (s - h + 1)^2 = (h + p - 1)^2, so the lp chain costs the same.
  - c as uint8 fixed-point round(c*255) [512, 512]; the 1/255 decode
    scale is folded into the S-chain first-pass copy-out scalar
    (-1/255 instead of -1), so decode costs zero extra device ops.
    Payload: 3 B/px = 6.3 MB/call instead of 31.5 MB, shipped as ONE
    u8 [512, 1536] array per core (s bytes 0:1024 via a DRAM-AP bf16
    bitcast, c bytes 1024:1536) to minimize per-buffer RPC overhead.
  - the 10.5 MB `bands` constant is device-resident (device_put once).
  - ONE shard_map jit executable, built and cached on first call (the
    stock run_bass_kernel_spmd axon path re-jits and re-ships a fresh
    executable every call).
  - output is fetched with jax.device_get immediately after async
    dispatch so the D2H overlaps the execute RPC.

Data parallel: core c handles images [c*IMGS, (c+1)*IMGS) sequentially,
reusing one set of SBUF tiles; each image accumulates into its own 9
columns of the osum output.
"""

import numpy as np

B, HH, WW = 8, 512, 512
KSIZE = 11
SIGMA = 2.0
# The execute RPC costs ~11ms per mesh core (measured: 2-core mesh ~51ms,
# 8-core ~118ms at equal payload), while per-device h2d streams at ~70MB/s
# in parallel and the device kernel is only ~20us/image.  Minimizing
# floor(n) + transfer(6.3MB/n) lands at n=4 cores x 2 images each.
NCORES = 4
IMGS = B // NCORES  # images processed sequentially per core
NBLK = 4  # 512 / 128
WIN = (0, 118, 246, 256)  # psum col window start per k-block, width 256
# rhs column offset inside the per-matrix band block (k0 / interior / k3 tiles)
BOFF = (0, 256, 256, 512)
# bands layout: zero 0:256 | B_hi 256:1024 | A 1024:1792 | B_lo 1792:2560
AOFF = (1024,)
BOFFS = (256, 1792)
ZCOL = 0
BANDS_W = 2560


def _gauss1d():
    ax = np.arange(KSIZE, dtype=np.float64) - (KSIZE - 1) / 2.0
    g = np.exp(-(ax ** 2) / (2.0 * SIGMA ** 2))
    return g / g.sum()


def _k2():
    gn = _gauss1d()
    k2d = np.outer(gn, gn)
    return float(np.sum(k2d * k2d))


def _round_m11(x):
    """Round f32 array to the PE's f32r grid (e10m11, HW-probed) nearest-even."""
    x = np.ascontiguousarray(x, dtype=np.float32)
    u = x.view(np.uint32).copy()
    u = (u + np.uint32(0x7FF) + ((u >> np.uint32(12)) & np.uint32(1))) & np.uint32(
        0xFFFFF000
    )
    out = u.view(np.float32).copy()
    out[x == 0.0] = 0.0
    return out


_np_cache = {}


def _bands_np():
    """[128, 2560] f32: zero | B_hi | A | B_lo, window tiles of 256 cols each."""
    if "bands" in _np_cache:
        return _np_cache["bands"]
    gn = _gauss1d()
    half = KSIZE // 2
    A = np.zeros((512, 512), dtype=np.float64)
    for o in range(-half, half + 1):
        idx = np.arange(max(0, -o), min(512, 512 - o))
        A[idx, idx + o] = gn[o + half]
    Bm = A @ A  # edge-exact double-conv matrix, band halfwidth 10

    def tiles(M):
        t = [M[128 * k: 128 * k + 128, WIN[k]: WIN[k] + 256] for k in range(4)]
        assert np.allclose(t[1], t[2], rtol=0, atol=1e-12), (
            "interior Toeplitz tiles must match"
        )
        return np.concatenate([t[0], t[1], t[3]], axis=1)

    At = tiles(A).astype(np.float32)
    Bt64 = tiles(Bm)
    B_hi = _round_m11(Bt64)
    B_lo = _round_m11(Bt64 - B_hi.astype(np.float64))
    zero = np.zeros((128, 256), dtype=np.float32)
    bands = np.concatenate([zero, B_hi, At, B_lo], axis=1)
    assert bands.shape == (128, BANDS_W)
    _np_cache["bands"] = np.ascontiguousarray(bands)
    return _np_cache["bands"]


_module_cache = {}


def _build_module(simsafe=None):
    import os

    if simsafe is None:
        simsafe = bool(os.environ.get("TRN_SIMSAFE"))
    key = ("nc", simsafe, NCORES)
    if key in _module_cache:
        return _module_cache[key]
    from contextlib import ExitStack

    import concourse.bass as bass  # noqa: F401
    import concourse.mybir as mybir
    import concourse.tile as tile
    from concourse import bacc

    f32 = mybir.dt.float32
    f32r = mybir.dt.float32r
    Alu = mybir.AluOpType
    Fn = mybir.ActivationFunctionType

    nc = bacc.Bacc("TRN2", target_bir_lowering=False, debug=False)

    bf16 = mybir.dt.bfloat16
    u8 = mybir.dt.uint8
    # merged input, IMGS images stacked on rows: per row, bytes 0:1024 = s
    # as bf16, 1024:1536 = c u8
    x_d = nc.dram_tensor("x_in", [IMGS * 512, 1536], u8, kind="ExternalInput")
    bands_d = nc.dram_tensor("bands", [128, BANDS_W], f32r, kind="ExternalInput")
    out_d = nc.dram_tensor("osum", [128, 9 * IMGS], f32, kind="ExternalOutput")

    with tile.TileContext(nc) as tc, ExitStack() as ctx:
        sb = ctx.enter_context(tc.tile_pool(name="sb", bufs=1))
        ps = ctx.enter_context(tc.tile_pool(name="ps", bufs=8, space="PSUM"))

        s_sb = sb.tile([128, 2048], f32r, name="s_sb")
        c_sb = sb.tile([128, 2048], f32r, name="c_sb")
        h_sb = sb.tile([128, 2048], f32r, name="h_sb")
        bands_sb = sb.tile([128, BANDS_W], f32r, name="bands_sb")
        t1_sb = sb.tile([128, 2048], f32r, name="t1_sb")
        s1_sb = sb.tile([128, 2048], f32r, name="s1_sb")
        d_sb = sb.tile([128, 2048], f32, name="d_sb")
        ab_sb = sb.tile([128, 2048], f32, name="ab_sb")
        lp_sb = sb.tile([128, 2048], f32, name="lp_sb")
        g_sb = sb.tile([128, 2048], f32, name="g_sb")
        mt_sb = sb.tile([128, 2048], f32, name="mt_sb")
        sums = sb.tile([128, 9 * IMGS], f32, name="sums")
        pos_one = sb.tile([128, 1], f32, name="pos_one")
        warm = sb.tile([1, 16], f32, name="warm")
        warm2 = sb.tile([1, 16], f32, name="warm2")

        # --- one-time setup ----------------------------------------------
        nc.gpsimd.memset(warm[:], 1.0)
        nc.scalar.activation(warm2[:], warm[:], Fn.Ln)
        nc.gpsimd.memset(pos_one[:], 1.0)

        if simsafe:
            nc.sync.dma_start(out=bands_sb[:, 0:768], in_=bands_d[:, 0:768])
        else:
            nc.sync.dma_start(out=bands_sb[:, 256:768], in_=bands_d[:, 256:768])
        nc.sync.dma_start(out=bands_sb[:, 768:1024], in_=bands_d[:, 768:1024])
        nc.sync.dma_start(out=bands_sb[:, 1792:2560], in_=bands_d[:, 1792:2560])
        nc.sync.dma_start(out=bands_sb[:, 1024:1792], in_=bands_d[:, 1024:1792])

        zero256 = bands_sb[:, ZCOL: ZCOL + 256]

        def conv_pass(src, mat_offs, out_tiles, init=True, fini=True, order="kb"):
            """out[ib] = src^T M banded: 4 kb-groups x 4 banks.

            mat_offs: one or two rhs column bases (hi, lo coefficient splits);
            multiple offsets accumulate into the same psum windows and share
            the stationary operand (no extra LDWEIGHTS).
            """
            last_off = mat_offs[-1]
            for j, mo in enumerate(mat_offs):
                loop = (
                    [(kb, ib) for kb in range(4) for ib in range(4)]
                    if order == "kb"
                    else [(kb, ib) for ib in range(4) for kb in range(4)]
                )
                for kb, ib in loop:
                    rhs = bands_sb[:, mo + BOFF[kb]: mo + BOFF[kb] + 256]
                    lhsT = src[:, 512 * kb + 128 * ib: 512 * kb + 128 * ib + 128]
                    nc.tensor.matmul(
                        out_tiles[ib][:, WIN[kb]: WIN[kb] + 256],
                        lhsT,
                        rhs,
                        start=(kb == 0 and j == 0 and init),
                        stop=(kb == 3 and mo == last_off and fini),
                    )
                    if simsafe and kb == 0 and j == 0 and init:
                        # CoreSim's per-bank pending-zero model needs every
                        # element TensorE-written before partial-window
                        # accumulation; on HW the four windows self-cover.
                        nc.tensor.matmul(
                            out_tiles[ib][:, 256:512],
                            lhsT,
                            zero256,
                            start=False,
                            stop=False,
                        )

        # --- per-image pipeline (sequential, shared tiles) -----------------
        f32 = mybir.dt.float32
        for b in range(IMGS):
            r0 = 512 * b
            a0 = 9 * b
            # input DMAs: s bf16 -> f32r and c u8 -> f32r cast-DMAs (SWDGE)
            for k in range(4):
                nc.gpsimd.dma_start(
                    out=s_sb[:, 512 * k: 512 * (k + 1)],
                    in_=x_d[r0 + 128 * k: r0 + 128 * (k + 1), 0:1024].bitcast(bf16),
                )
            for k in range(4):
                nc.gpsimd.dma_start(
                    out=c_sb[:, 512 * k: 512 * (k + 1)],
                    in_=x_d[r0 + 128 * k: r0 + 128 * (k + 1), 1024:1536],
                )
            # decode h = (s > 0) per 512-col block (vector engine)
            for k in range(4):
                sl = slice(512 * k, 512 * (k + 1))
                nc.vector.tensor_scalar(
                    h_sb[:, sl], s_sb[:, sl], 0.0, None, Alu.is_gt
                )

            # T chain: T2 = B h B
            tT1 = [
                ps.tile([128, 512], f32, name=f"tT1_{b}_{i}", tag="bank")
                for i in range(4)
            ]
            conv_pass(h_sb, BOFFS, tT1)
            for ib in range(4):
                dst = t1_sb[:, 512 * ib: 512 * (ib + 1)]
                if ib % 2 == 0:
                    nc.vector.tensor_copy(dst, tT1[ib][:])
                else:
                    nc.scalar.copy(dst, tT1[ib][:])
            # S chain first pass: S1 = c^T A (negated on copy-out);
            # copy-out scale -1/255 folds the u8 fixed-point decode of c
            tS1 = [
                ps.tile([128, 512], f32, name=f"tS1_{b}_{i}", tag="bank")
                for i in range(4)
            ]
            conv_pass(c_sb, AOFF, tS1)
            for ib in range(4):
                dst = s1_sb[:, 512 * ib: 512 * (ib + 1)]
                if ib % 2 == 0:
                    nc.vector.tensor_scalar(
                        dst, tS1[ib][:], -1.0 / 255.0, None, Alu.mult
                    )
                else:
                    nc.scalar.mul(dst, tS1[ib][:], -1.0 / 255.0)

            # second passes: corr = t1^T B - s1^T A into shared banks
            tT2 = [
                ps.tile([128, 512], f32, name=f"tT2_{b}_{i}", tag="bank")
                for i in range(4)
            ]
            conv_pass(t1_sb, BOFFS, tT2, init=True, fini=False)
            conv_pass(s1_sb, AOFF, tT2, init=False, fini=True, order="ib")

            # lp chain (last block in halves to shorten the tail)
            lp_parts = [(ib, 512 * ib, 512) for ib in range(3)]
            lp_parts += [(3, 1536, 256), (3, 1792, 256)]
            lp_acc_col = [4, 5, 6, 7, 8]
            for idx, (ib, s0, w) in enumerate(lp_parts):
                s = slice(s0, s0 + w)
                hv = h_sb[:, s].bitcast(f32)
                sv = s_sb[:, s].bitcast(f32)
                # d = s - h
                nc.gpsimd.tensor_tensor(d_sb[:, s], sv, hv, Alu.subtract)
                # a = (d + 1)^2 = (h + p - 1)^2   (in [1e-4, 1])
                nc.scalar.activation(
                    ab_sb[:, s], d_sb[:, s], Fn.Square, bias=pos_one[:]
                )
                # lp2 = ln(a) = 2*lp, accumulate per-partition sum(2*lp)
                col = a0 + lp_acc_col[idx]
                nc.scalar.activation(
                    lp_sb[:, s], ab_sb[:, s], Fn.Ln,
                    accum_out=sums[:, col: col + 1],
                )
                # gt2 = (h - 0.5) * lp2  ( = -delta*lp )
                nc.vector.scalar_tensor_tensor(
                    g_sb[:, s], hv, 0.5, lp_sb[:, s], Alu.subtract, Alu.mult
                )

            # final products + accumulation
            for ib in range(4):
                s = slice(512 * ib, 512 * (ib + 1))
                nc.vector.scalar_tensor_tensor(
                    mt_sb[:, s], tT2[ib][:], 0.25, g_sb[:, s], Alu.mult, Alu.mult,
                    accum_out=sums[:, a0 + ib: a0 + ib + 1],
                )

        nc.sync.dma_start(out=out_d[:], in_=sums[:])

    nc.finalize()
    _module_cache[key] = nc
    return nc


_pack_bufs = {}


def _pack_x(prob_map, c, h_sampled):
    """Returns x: global (4096, 1536) u8. Per row, bytes 0:1024 hold s =
    sign(h-0.5)*p rounded to bf16; bytes 1024:1536 hold round(c*255) u8."""
    if not _pack_bufs:
        _pack_bufs["f"] = np.empty((B * 512, 512), np.float32)
        _pack_bufs["x"] = np.empty((B * 512, 1536), np.uint8)
    p = prob_map.reshape(B * 512, 512)
    cc = c.reshape(B * 512, 512)
    h = h_sampled.reshape(B * 512, 512)
    x = _pack_bufs["x"]

    f = _pack_bufs["f"]
    np.subtract(h, np.float32(0.5), out=f)   # ±0.5 carrying h in the sign
    np.copysign(p, f, out=f)                 # s = ±p  (sign = h)
    u = f.view(np.uint32)
    u += np.uint32(0x8000)                   # bf16 round-half-up on |s|
    u >>= np.uint32(16)
    xs16 = x[:, 0:1024].view(np.uint16)
    xs16[:] = u                              # downcast copy u32 -> u16

    np.multiply(cc, np.float32(255.0), out=f)
    f += np.float32(0.5)
    x[:, 1024:1536] = f                      # truncating downcast = rounding
    return x


def _sim_map(prob_map, c, h_sampled, core):
    """Per-core input map for CoreSim (core handles images
    [core*IMGS, (core+1)*IMGS))."""
    x = _pack_x(prob_map, c, h_sampled)
    return {
        "x_in": np.ascontiguousarray(
            x[IMGS * 512 * core: IMGS * 512 * (core + 1)]
        ),
        "bands": _bands_np(),
    }


def _reduce_host(osums):
    """osums: iterable of per-core (128, 9*IMGS) arrays."""
    k2 = _k2()
    total = 0.0
    for o in osums:
        o = np.asarray(o, dtype=np.float64).reshape(128, IMGS, 9)
        total += o[:, :, 0:4].sum() - (k2 / 16.0) * o[:, :, 4:9].sum()
    return np.float32(total)


_rt = {}


def _init_runtime():
    if _rt:
        return _rt
    import jax
    import concourse.mybir as mybir
    from concourse.bass2jax import (
        _bass_exec_p,
        install_neuronx_cc_hook,
        partition_id_tensor,
    )
    from jax.sharding import Mesh, NamedSharding, PartitionSpec
    from jax.experimental.shard_map import shard_map

    install_neuronx_cc_hook()
    nc = _build_module(simsafe=False)

    in_names, out_names, out_avals, zero_shapes = [], [], [], []
    partition_name = (
        nc.partition_id_tensor.name if nc.partition_id_tensor else None
    )
    for alloc in nc.m.functions[0].allocations:
        if not isinstance(alloc, mybir.MemoryLocationSet):
            continue
        name = alloc.memorylocations[0].name
        if alloc.kind == "ExternalInput":
            if name != partition_name:
                in_names.append(name)
        elif alloc.kind == "ExternalOutput":
            out_names.append(name)
            shape = tuple(alloc.tensor_shape)
            dtype = mybir.dt.np(alloc.dtype)
            out_avals.append(jax.core.ShapedArray(shape, dtype))
            zero_shapes.append(((NCORES * shape[0], *shape[1:]), dtype))

    n_params = len(in_names)
    n_outs = len(out_avals)
    in_names_all = list(in_names) + list(out_names)
    if partition_name is not None:
        in_names_all.append(partition_name)
    donate = tuple(range(n_params, n_params + n_outs))

    def _body(*args):
        operands = list(args)
        if partition_name is not None:
            operands.append(partition_id_tensor())
        return tuple(
            _bass_exec_p.bind(
                *operands,
                out_avals=tuple(out_avals),
                in_names=tuple(in_names_all),
                out_names=tuple(out_names),
                lowering_input_output_aliases=(),
                sim_require_finite=True,
                sim_require_nnan=True,
                nc=nc,
            )
        )

    devices = jax.devices()[:NCORES]
    mesh = Mesh(np.asarray(devices), ("core",))
    fn = jax.jit(
        shard_map(
            _body,
            mesh=mesh,
            in_specs=(PartitionSpec("core"),) * (n_params + n_outs),
            out_specs=(PartitionSpec("core"),) * n_outs,
            check_rep=False,
        ),
        donate_argnums=donate,
        keep_unused=True,
    )

    sh = NamedSharding(mesh, PartitionSpec("core"))
    bands_global = np.broadcast_to(
        _bands_np()[None], (NCORES, 128, BANDS_W)
    ).reshape(NCORES * 128, BANDS_W)
    try:
        bands_dev = jax.device_put(np.ascontiguousarray(bands_global), sh)
        bands_dev.block_until_ready()
    except jax.errors.JaxRuntimeError:
        # transient relay/device hiccup — one retry after a short pause
        import time as _time

        _time.sleep(2.0)
        bands_dev = jax.device_put(np.ascontiguousarray(bands_global), sh)
        bands_dev.block_until_ready()

    # in_names order is declaration order: x_in, bands
    assert in_names == ["x_in", "bands"], in_names
    assert out_names == ["osum"], out_names

    _rt.update(
        nc=nc,
        fn=fn,
        bands_dev=bands_dev,
        zero_shapes=zero_shapes,
        # donation consumes the per-call device buffers, not these host
        # arrays, so they are safely reusable across calls
        zeros=[np.zeros(shape, dt) for shape, dt in zero_shapes],
        out_shape=tuple(out_avals[0].shape),
    )
    return _rt


def kernel(prob_map, c, h_sampled, **kw_extra):
    import time as _time

    import jax

    rt = _init_runtime()
    x = _pack_x(
        np.asarray(prob_map, dtype=np.float32),
        np.asarray(c, dtype=np.float32),
        np.asarray(h_sampled, dtype=np.float32),
    )
    # transient relay/device hiccups (NRT_EXEC_UNIT_UNRECOVERABLE after
    # executable switches) sometimes clear on retry — back off and reattempt
    for attempt, pause in ((0, 3.0), (1, 10.0), (2, None)):
        try:
            out = rt["fn"](x, rt["bands_dev"], *rt["zeros"])[0]
            host = jax.device_get(out)
            break
        except jax.errors.JaxRuntimeError:
            if pause is None:
                raise
            _time.sleep(pause)
    return _reduce_host(host.reshape(NCORES, *rt["out_shape"]))

